# revision 44
# baseline (speedup 1.0000x reference)
"""Multi-head self-attention (B=2, S=2048, D=1024, H=16, causal) on 8 NeuronCores.

Sharding: 32 (batch, head) instances -> 4 heads of one batch per core
(cores 0-3: batch 0, cores 4-7: batch 1; core c owns heads 4*(c%4) .. +3).
Wq/Wk/Wv are split by rows (head dims), Wo by columns; each core computes a
partial y[b] = attn_out_heads @ Wo_cols.T and the host sums the 4 partials
per batch at gather time (tensor-parallel reduce).

Per-core kernel. All matmuls fp16 x fp16 -> fp32 psum. No on-device
transposes:
  QT[256,2048] = wqT.T @ xT        (head-pair tiles: rows 0-63 / 64-127)
  KT likewise; V[2048,256] natural (lhsT = xT chunks), augmented with a
  ones column per head -> va tiles [128, 4*65] (ones via memset).
  Scores computed transposed, blockwise [k-tile 128, q-chunk 512]:
      S^T = KT_h.T @ QT_h   -- two heads row-packed (contraction d=64 at
      partition bases 0 / 64). Both heads of a k-tile share one [128,1024]
      psum tile (bufs=2 -> exp overlaps the next k-tile's score matmuls);
      one Exp (scale=1/8) per tile on ScalarE -> P fp16 in SBUF.
  Causal masking only on diagonal k-tiles via affine_select (no mask
  tensor or mask DMA needed).
  AV: lhsT = [V_h | 1] fp16 [k,65], rhs = P [k,512] -> psum [65,512]
      accumulated over k-tiles = unnormalized out^T (rows 0-63) + softmax
      denominators (row 64). Normalize columns via reciprocal_approx_fast +
      matmul partition broadcast + DVE multiply -> out_headsT [256,2048] fp16.
  y = out_headsT.T @ woT -> [2048, 1024] fp16 partial, DMA'd out
      (host accumulates partials in fp32).

v2 changes vs the 173.5us baseline (all aimed at keeping the PE stream
continuous; the kernel is matmul-column bound at ~278k columns):
  - Input DMAs split fine (64KB) and spread across sync/vector/gpsimd/
    scalar queues in first-use order, so the first projection matmul can
    start at ~4-5us instead of 10us (per-dma_start queue issue overhead
    ~0.6us serialized per queue was the gate, and Tile tracks region-level
    deps so each accumulation matmul only waits for its own block).
  - PSUM evacuations spread across engines: QT/KT copies on ACT+DVE,
    V copies on GPS(Pool), ys copies alternate DVE/GPS (final chunk also
    ACT) -- the baseline put all of them on DVE (95us busy) which stalled
    the PE at every chunk boundary.
  - Causal masking via affine_select on DVE (kills the 0.5MB mask DMA and
    the sync-queue issue slots it occupied).
  - Final chunk: norm_s2(3,0) emitted before kloop(3,1) so only hp1's
    normalize + wo remain on the tail.

Pipeline over q-chunks. The next chunk's QKV projections are emitted after
chunk n's two attention k-loops but before hp1's normalize + output
projection, so the tensor engine stays busy through the normalize tail.
Projections are never interleaved with attention's score/AV psum
accumulation groups (nondeterministic hardware corruption when they are;
verified on HW).
"""
import os
import sys

sys.path.insert(0, "/opt/trn_rl_repo")

import numpy as np

import concourse.bass as bass  # noqa: F401
import concourse.mybir as mybir
from concourse import bacc
from concourse.tile import TileContext
from concourse.bass_utils import run_bass_kernel_spmd

B, S, D = 2, 2048, 1024
H, HD = 16, 64
NCORES = 8
HPC = 4            # heads per core
SC = 512           # q-chunk width
NQC = S // SC      # 4 q-chunks
NKT = S // 128     # 16 k-tiles
F16 = mybir.dt.float16
F32 = mybir.dt.float32
ATTN_SCALE = 1.0 / np.sqrt(HD)

_CACHE = {}


def _build():
    nc = bacc.Bacc("TRN2", target_bir_lowering=False, debug=False, num_devices=NCORES)

    xT_d = nc.declare_dram_parameter("xT", [D, S], F16, isOutput=False)
    wqT_d = nc.declare_dram_parameter("wqT", [D, 256], F16, isOutput=False)
    wkT_d = nc.declare_dram_parameter("wkT", [D, 256], F16, isOutput=False)
    wvT_d = nc.declare_dram_parameter("wvT", [D, 256], F16, isOutput=False)
    woT_d = nc.declare_dram_parameter("woT", [256, D], F16, isOutput=False)
    y_d = nc.declare_dram_parameter("y", [S, D], F16, isOutput=True)

    with TileContext(nc) as tc:
        with (
            tc.tile_pool(name="static", bufs=1) as st,
            tc.tile_pool(name="ppool", bufs=10) as ppool,
            tc.tile_pool(name="rbpool", bufs=6) as rbpool,
            tc.tile_pool(name="recpool", bufs=4) as recpool,
            tc.tile_pool(name="ystage", bufs=4) as ystage,
            tc.tile_pool(name="psA", bufs=2, space="PSUM") as psA,
            tc.tile_pool(name="psS", bufs=2, space="PSUM") as psS,
            tc.tile_pool(name="psV", bufs=1, space="PSUM") as psV,
        ):
            wq = st.tile([128, 2048], F16, name="wq", tag="wq")
            wk = st.tile([128, 2048], F16, name="wk", tag="wk")
            wv = st.tile([128, 2048], F16, name="wv", tag="wv")
            wo = st.tile([128, 2048], F16, name="wo", tag="wo")
            xT = [st.tile([128, S], F16, name=f"xT{k}", tag=f"xT{k}") for k in range(8)]
            ones64 = st.tile([1, 64], F16, name="ones64", tag="ones64")
            va = [
                st.tile([128, 65 * HPC], F16, name=f"va{i}", tag=f"va{i}")
                for i in range(NKT)
            ]

            # ---- input DMAs, split fine (64KB) and spread across the three
            # DMA-capable queues (sync/SP, scalar/ACT, gpsimd/Pool) in
            # first-use order so the projection matmuls stream against DMA
            # arrivals (Tile tracks region-level deps, so each accumulation
            # matmul only waits for its own 128-row block).
            #   sync:   xT chunk0 (8 x 128KB), then xT chunks 1-3
            #   gpsimd: all weights (SWDGE issues are ~4x cheaper than the
            #           sync queue's HWDGE config), va-ones memsets, wo
            #   scalar: no input DMAs (keeps the ACT queue free for copies)
            #   vector: ones64 memset (no DMAs allowed on DVE)
            nc.vector.memset(ones64[:], 1.0)
            warm = st.tile([1, SC], F16, name="warm", tag="warm")
            nc.vector.memset(warm[:], 0.0)
            for k in range(8):
                for h2 in range(2):
                    nc.sync.dma_start(
                        out=xT[k][:, 256 * h2 : 256 * h2 + 256],
                        in_=xT_d[128 * k : 128 * k + 128, 256 * h2 : 256 * h2 + 256],
                    )
            for k in range(8):
                nc.gpsimd.dma_start(
                    out=wq[:, 256 * k : 256 * k + 256],
                    in_=wqT_d[128 * k : 128 * k + 128, :],
                )
            for k in range(8):
                nc.scalar.dma_start(
                    out=wk[:, 256 * k : 256 * k + 256],
                    in_=wkT_d[128 * k : 128 * k + 128, :],
                )
            for k in range(8):
                nc.gpsimd.dma_start(
                    out=wv[:, 256 * k : 256 * k + 256],
                    in_=wvT_d[128 * k : 128 * k + 128, :],
                )
            # ones columns of the va tiles (position 64 of each head slot)
            for i in range(NKT):
                ones_ap = va[i].rearrange("p (h c) -> p h c", c=65)[:, :, 64]
                nc.gpsimd.memset(ones_ap, 1.0)
            # causal boundary mask, generated on-device: trimask[p, j] = 1
            # iff j >= p (keep query j >= key p within a diagonal block)
            trimask = st.tile([128, 128], F16, name="trimask", tag="trimask")
            nc.gpsimd.memset(trimask[:], 1.0)
            nc.gpsimd.affine_select(
                trimask[:],
                trimask[:],
                pattern=[[1, 128]],
                compare_op=mybir.AluOpType.is_ge,
                fill=0.0,
                base=0,
                channel_multiplier=-1,
            )
            for cc in range(2):
                for h2 in range(2):
                    nc.gpsimd.dma_start(
                        out=wo[:, 1024 * cc + 512 * h2 : 1024 * cc + 512 * h2 + 512],
                        in_=woT_d[128 * cc : 128 * cc + 128, 512 * h2 : 512 * h2 + 512],
                    )
            # wo arrives well before wo_chunk(0) at ~35us
            for n in range(1, NQC):
                for k in range(8):
                    nc.sync.dma_start(
                        out=xT[k][:, SC * n : SC * n + SC],
                        in_=xT_d[128 * k : 128 * k + 128, SC * n : SC * n + SC],
                    )

            # throwaway exp so the ~2.7us exp_and_others ACT table load
            # happens during the projection phase instead of serially in
            # front of chunk 0's first real exp. Output is never read.
            expwarm = st.tile([1, 8], F16, name="expwarm", tag="expwarm")
            nc.scalar.activation(
                expwarm[:],
                ones64[0:1, 0:8],
                mybir.ActivationFunctionType.Exp,
                scale=1.0,
            )

            # PE p-state warm-up: the tensor engine ramps 0.65 -> 1.2 ->
            # 2.4GHz only after ~3us of continuous execution; burn dummy
            # matmuls on a zero tile (into dead psum) during the input DMA
            # phase so the clock is hot when chunk-0 data lands.
            for _w in range(20):
                warmps = psA.tile([64, SC], F32, name="warmps", tag="acc")
                nc.tensor.matmul(
                    warmps[:], warm[0:1, 0:64], warm[:], start=True, stop=True
                )

            QT = [st.tile([128, S], F16, name=f"QT{m}", tag=f"QT{m}") for m in range(2)]
            KT = [st.tile([128, S], F16, name=f"KT{m}", tag=f"KT{m}") for m in range(2)]
            outT = [
                st.tile([128, S], F16, name=f"outT{m}", tag=f"outT{m}")
                for m in range(2)
            ]

            def proj_qk_chunk(n):
                # m-major so the m=0 (head-pair 0) tiles land first: the next
                # k-loop is kloop(n,0). KT copies on ACT, QT copies on DVE.
                for m in range(2):
                    for dst, w, cpeng in ((KT, wk, nc.scalar), (QT, wq, nc.vector)):
                        acc = psA.tile([128, SC], F32, name="acc", tag="acc")
                        for k in range(8):
                            nc.tensor.matmul(
                                acc[:],
                                w[:, 256 * k + 128 * m : 256 * k + 128 * m + 128],
                                xT[k][:, SC * n : SC * n + SC],
                                start=(k == 0),
                                stop=(k == 7),
                            )
                        if cpeng is nc.scalar:
                            cpeng.copy(dst[m][:, SC * n : SC * n + SC], acc[:])
                        else:
                            cpeng.tensor_copy(dst[m][:, SC * n : SC * n + SC], acc[:])

            def proj_v(i):
                accv = psA.tile([128, 256], F32, name="accv", tag="acc")
                for k in range(8):
                    nc.tensor.matmul(
                        accv[:],
                        xT[k][:, 128 * i : 128 * i + 128],
                        wv[:, 256 * k : 256 * k + 256],
                        start=(k == 0),
                        stop=(k == 7),
                    )
                for h in range(HPC):
                    # GPSIMD can't read PSUM; split the evacuations between
                    # DVE and ACT so neither engine owns the whole burst
                    if h % 2 == 0:
                        nc.vector.tensor_copy(
                            va[i][:, 65 * h : 65 * h + 64],
                            accv[:, 64 * h : 64 * h + 64],
                        )
                    else:
                        nc.scalar.copy(
                            va[i][:, 65 * h : 65 * h + 64],
                            accv[:, 64 * h : 64 * h + 64],
                        )

            def attn_kloop(jq, hp, qoff=0, qw=SC):
                """Scores+exp+AV for one head pair over q-columns
                [SC*jq+qoff, SC*jq+qoff+qw); av psum tiles returned
                unnormalized (rows 0-63 = out^T, row 64 = denominators)."""
                qbase = SC * jq + qoff
                nkt = (qbase + qw) // 128  # causal: k <= last q
                # always allocate full-size psum tiles (a smaller tile could
                # share a PSUM bank and matmul start=True zeroing is
                # bank-granular); slice views for sub-chunk widths
                av = [
                    psV.tile([65, SC], F32, name=f"av{u}", tag=f"av{u}")
                    for u in range(2)
                ]
                for kt in range(nkt):
                    # causal trim: q-columns < 128*kt - qbase are all-masked;
                    # skip them in scores/exp/AV.
                    off = min(max(128 * kt - qbase, 0), qw)
                    diag = 128 * kt + 127 > qbase + off  # partially-masked blk
                    # NOTE: the u-halves keep their full-width SC stride even
                    # for qw<SC sub-chunks, so the two heads' score
                    # accumulation groups stay in separate PSUM banks (two
                    # concurrently-open groups in one 2KB bank hang the HW).
                    sp = psS.tile([128, 2 * SC], F32, name="sp", tag="sp")
                    for u, base in ((0, 0), (1, 64)):
                        nc.tensor.matmul(
                            sp[:, SC * u + off : SC * u + qw],
                            KT[hp][base : base + 64, 128 * kt : 128 * kt + 128],
                            QT[hp][base : base + 64, qbase + off : qbase + qw],
                            start=True,
                            stop=True,
                        )
                    pt = ppool.tile([128, 2 * SC], F16, name="pt", tag="pt")
                    sp3 = sp.rearrange("p (u q) -> p u q", u=2)[:, :, off:qw]
                    pt3 = pt.rearrange("p (u q) -> p u q", u=2)[:, :, off:qw]
                    nc.scalar.activation(
                        pt3,
                        sp3,
                        mybir.ActivationFunctionType.Exp,
                        scale=float(ATTN_SCALE),
                    )
                    if diag:
                        # only the boundary block is partially masked (cols
                        # < off were trimmed; cols >= off+128 are fully
                        # below the diagonal). GPS (Pool) handles it: it is
                        # SBUF->SBUF, and this keeps DVE free for the psum
                        # evacuations that gate the chunk boundaries.
                        bw = min(128, qw - off)
                        for u in range(2):
                            blk = slice(SC * u + off, SC * u + off + bw)
                            nc.gpsimd.tensor_mul(
                                pt[:, blk], pt[:, blk], trimask[:, 0:bw]
                            )
                    for u in range(2):
                        h = 2 * hp + u
                        nc.tensor.matmul(
                            av[u][:, off:qw],
                            va[kt][:, 65 * h : 65 * h + 65],
                            pt[:, SC * u + off : SC * u + qw],
                            start=(kt == 0),
                            stop=(kt == nkt - 1),
                        )
                return av

            # normalize columns by softmax denominators (row 64), split in
            # two stages so other work can be emitted between them:
            # stage 1 (DVE only): evacuate av to SBUF (this read is what
            # frees the av psum slots for the next k-loop) and compute the
            # fp16 reciprocal row. stage 2 (1 matmul + 1 DVE mul per head):
            # broadcast 1/den across 64 partitions via a K=1 matmul against
            # a ones row, multiply.
            def attn_norm_s1(av, qw=SC, split=False):
                # evacuate both heads first: these two copies are what free
                # the av psum slots, and the next k-loop's ops queue behind
                # them -- keep the (longer) reciprocal chains after both
                # copies. split=True (used on the tail where ACT has no
                # more exps) runs u=1 on ACT in parallel.
                avss = []
                for u in range(2):
                    avs = rbpool.tile([65, SC], F32, name="avs", tag="avs")
                    if split and u == 1:
                        nc.scalar.copy(avs[:, 0:qw], av[u][:, 0:qw])
                    else:
                        nc.vector.tensor_copy(avs[:, 0:qw], av[u][:, 0:qw])
                    avss.append(avs)
                stg = []
                for u in range(2):
                    den = recpool.tile([1, SC], F32, name="den", tag="den")
                    nc.vector.tensor_copy(den[:, 0:qw], avss[u][64:65, 0:qw])
                    rec = recpool.tile([1, SC], F32, name="rec", tag="rec")
                    nc.vector.reciprocal_approx_fast(rec[:, 0:qw], den[:, 0:qw])
                    rec16 = recpool.tile([1, SC], F16, name="rec16", tag="rec16")
                    nc.vector.tensor_copy(rec16[:, 0:qw], rec[:, 0:qw])
                    stg.append((avss[u], rec16))
                return stg

            def attn_norm_s2(jq, hp, stg, qoff=0, qw=SC):
                qbase = SC * jq + qoff
                for u, (avs, rec16) in enumerate(stg):
                    rbp = psA.tile([64, SC], F32, name="rbp", tag="acc")
                    nc.tensor.matmul(
                        rbp[:, 0:qw],
                        ones64[0:1, :],
                        rec16[:, 0:qw],
                        start=True,
                        stop=True,
                    )
                    nc.vector.tensor_mul(
                        outT[hp][64 * u : 64 * u + 64, qbase : qbase + qw],
                        avs[0:64, 0:qw],
                        rbp[:, 0:qw],
                    )

            def attn_normalize(jq, hp, av):
                attn_norm_s2(jq, hp, attn_norm_s1(av))

            def wo_chunk(jq, i_range=None, last=False):
                if i_range is None:
                    i_range = range(4 * jq, 4 * jq + 4)
                for idx, i in enumerate(i_range):
                    for n in range(2):
                        yp = psA.tile([128, 512], F32, name="yp", tag="acc")
                        for cc in range(2):
                            nc.tensor.matmul(
                                yp[:],
                                outT[cc][:, 128 * i : 128 * i + 128],
                                wo[:, 1024 * cc + 512 * n : 1024 * cc + 512 * n + 512],
                                start=(cc == 0),
                                stop=(cc == 1),
                            )
                        ys = ystage.tile([128, 512], F16, name="ys", tag="ys")
                        j = 2 * idx + n
                        # GPSIMD can't read PSUM; alternate DVE/ACT. On the
                        # final chunk the copy latency is the tail's critical
                        # chain (psA bufs=2), so split each copy in half
                        # across both engines.
                        if last:
                            nc.vector.tensor_copy(ys[:, 0:256], yp[:, 0:256])
                            nc.scalar.copy(ys[:, 256:512], yp[:, 256:512])
                        elif j % 2 == 0:
                            nc.vector.tensor_copy(ys[:], yp[:])
                        else:
                            nc.scalar.copy(ys[:], yp[:])
                        if last:
                            eng = (nc.sync, nc.gpsimd, nc.scalar)[j % 3]
                        else:
                            eng = nc.sync
                        eng.dma_start(
                            out=y_d[128 * i : 128 * i + 128, 512 * n : 512 * n + 512],
                            in_=ys[:],
                        )

            # NOTE: projections must not interleave with attention's score/AV
            # psum accumulation groups (nondeterministic hardware corruption,
            # verified repeatedly on HW). Emitting proj(n+1) after chunk n's
            # k-loops but before its hp1 normalize + wo keeps the tensor
            # queue fed through the normalize tail without touching the
            # attention groups.
            # Software pipeline over chunks. Per chunk boundary the emission
            # is: [kloop(n,1)] [proj(n+1)+V] [norm(n,0) stage2]
            # [norm(n,1) stage1] [kloop(n+1,0)] [norm(n,1) stage2] [wo(n)].
            # The next chunk's first k-loop sits BEFORE chunk n's remaining
            # bcast/wo work, so the scalar engine's exp stream resumes as
            # soon as the projections land instead of also waiting out the
            # normalize broadcasts and the output projection. All psum
            # accumulation groups are closed at every insertion point.
            # Final chunk (no next proj/kloop): norm(3,0) runs before
            # kloop(3,1) so only hp1's normalize + wo remain on the tail.
            mode = os.environ.get("KV_PIPE", "2")
            if mode == "2":
                proj_qk_chunk(0)
                for i in range(4):
                    proj_v(i)
                av0 = attn_kloop(0, 0)
                for n in range(NQC):
                    s1_0 = attn_norm_s1(av0)
                    if n + 1 < NQC:
                        av1 = attn_kloop(n, 1)
                        proj_qk_chunk(n + 1)
                        for i in range(4 * n + 4, 4 * n + 8):
                            proj_v(i)
                        attn_norm_s2(n, 0, s1_0)
                        s1_1 = attn_norm_s1(av1)
                        av0 = attn_kloop(n + 1, 0)
                        attn_norm_s2(n, 1, s1_1)
                        wo_chunk(n)
                    else:
                        # final chunk: hp0's normalize runs before kloop(3,1)
                        # so only hp1's normalize + wo remain on the tail.
                        # (Splitting kloop(3,1) into 256-wide halves was
                        # tried: it shortens the tail ~2us but adds ~6us of
                        # per-instruction overhead in the finer k-loops.)
                        attn_norm_s2(n, 0, s1_0)
                        av1 = attn_kloop(n, 1)
                        s1_1 = attn_norm_s1(av1, split=True)
                        attn_norm_s2(n, 1, s1_1)
                        wo_chunk(n, last=True)
            elif mode == "1":
                proj_qk_chunk(0)
                for i in range(4):
                    proj_v(i)
                for n in range(NQC):
                    av0 = attn_kloop(n, 0)
                    attn_normalize(n, 0, av0)
                    av1 = attn_kloop(n, 1)
                    if n + 1 < NQC:
                        proj_qk_chunk(n + 1)
                        for i in range(4 * n + 4, 4 * n + 8):
                            proj_v(i)
                    attn_normalize(n, 1, av1)
                    wo_chunk(n)
            else:
                for n in range(NQC):
                    proj_qk_chunk(n)
                    for i in range(4 * n, 4 * n + 4):
                        proj_v(i)
                    av0 = attn_kloop(n, 0)
                    attn_normalize(n, 0, av0)
                    av1 = attn_kloop(n, 1)
                    attn_normalize(n, 1, av1)
                    wo_chunk(n)

    nc.compile()
    return nc


def kernel(x, Wq, Wk, Wv, Wo):
    x = np.asarray(x, dtype=np.float32)
    Wq = np.asarray(Wq, dtype=np.float32)
    Wk = np.asarray(Wk, dtype=np.float32)
    Wv = np.asarray(Wv, dtype=np.float32)
    Wo = np.asarray(Wo, dtype=np.float32)

    if "nc" not in _CACHE:
        _CACHE["nc"] = _build()
    nc = _CACHE["nc"]

    xT = [np.ascontiguousarray(x[b].T).astype(np.float16) for b in range(B)]
    in_maps = []
    for c in range(NCORES):
        b, g = c // 4, c % 4
        rows = slice(256 * g, 256 * g + 256)
        in_maps.append(
            {
                "xT": xT[b],
                "wqT": np.ascontiguousarray(Wq[rows].T).astype(np.float16),
                "wkT": np.ascontiguousarray(Wk[rows].T).astype(np.float16),
                "wvT": np.ascontiguousarray(Wv[rows].T).astype(np.float16),
                "woT": np.ascontiguousarray(Wo[:, rows].T).astype(np.float16),
            }
        )

    trace = False
    if os.environ.get("KERNEL_TRACE") == "1":
        try:
            from trn_agent_boot.trn_boot import _ntff_profile_via_ctypes

            try:
                from antenv.axon_hooks import (
                    get_axon_ntff_profile_hook,
                    set_axon_ntff_profile_hook,
                )
            except ImportError:
                # this image's antenv lacks axon_hooks; provide the
                # 2-function registry bass_utils expects (test-only path)
                import types

                import antenv

                mod = types.ModuleType("antenv.axon_hooks")
                mod._hook = None

                def set_axon_ntff_profile_hook(h, _m=mod):
                    _m._hook = h

                def get_axon_ntff_profile_hook(_m=mod):
                    return _m._hook

                mod.set_axon_ntff_profile_hook = set_axon_ntff_profile_hook
                mod.get_axon_ntff_profile_hook = get_axon_ntff_profile_hook
                sys.modules["antenv.axon_hooks"] = mod
                antenv.axon_hooks = mod

            if get_axon_ntff_profile_hook() is None:
                set_axon_ntff_profile_hook(
                    _ntff_profile_via_ctypes("/opt/axon/libaxon_pjrt.so")
                )
            trace = True
        except Exception:
            trace = False

    res = run_bass_kernel_spmd(nc, in_maps, core_ids=list(range(NCORES)), trace=trace)
    _CACHE["exec_time_ns"] = res.exec_time_ns
    _CACHE["res"] = res
    y = np.zeros((B, S, D), dtype=np.float32)
    for c in range(NCORES):
        y[c // 4] += res.results[c]["y"].astype(np.float32)
    return y
